# revision 13
# baseline (speedup 1.0000x reference)
"""Trainium2 Bass kernel for nn_Attention_47682726920277.

Causal multi-head attention with RoPE:
  q/k/v = x @ w{q,k,v}.T ; RoPE(q, k) ; att = softmax(mask(q k^T / 8)) ; out = (att v) @ wo.T
Shapes: x [2, 2048, 1024], 16 heads of dim 64, fp32.

Sharding (8 cores): data-parallel over batch (2) x tensor-parallel over heads (4 per
core). Each core computes its 4 heads' attention and a partial out via its wo row
block; the final all-reduce is the host-side sum of the 4 partials per batch.

Design notes:
- The PE matmul stream (~115us of columns at 2.4GHz) and the scalar engine's
  softmax-exp stream (~66us) are the two long poles; projection/V/wo matmuls
  are interleaved between each k-block's score and PV matmuls at ~1.1k-cycle
  granularity so the PE never idles (idle also drops the PE to its half-rate
  DVFS p-state).
- Q/K channels are stored pair-interleaved per head ([0,32,1,33,...] of the
  natural order) so RoPE's rotate-half partner is the adjacent channel; the
  partner term is then a single vector stream_shuffle (a per-32-lane-quadrant
  crossbar) instead of 4 partition-window ops. Scores are invariant to the
  permutation since q and k share it.
- DMA triggers cost ~0.7us each, serialized per issuing queue: input loads are
  split between the sync and scalar queues and kept coarse; wo outputs are one
  trigger each.
- Scalar runs exp exclusively (one activation table, no swaps). Softmax
  reciprocals use the DVE fast-approx reciprocal; the per-column denominator
  broadcast and the normalize multiply run on gpsimd.
"""
import sys
import types
import numpy as np

B = 2
T = 2048
D = 1024
H = 16
HD = 64
NCORES = 8
GROUPS = NCORES // B          # head-groups per batch
HPC = H // GROUPS             # heads per core = 4
CH = HPC * HD                 # channels per core = 256
NQ = 512                      # PSUM bank width (fp32)
P = 128

KB_BUDGET = 1100              # PE cycles of stuffing pulled per attention k-block

_prog_cache = {}


def _install_ntff_shim():
    """The agent image's antenv lacks axon_hooks; inject it so trace=True works."""
    try:
        import antenv.axon_hooks  # noqa: F401
        return
    except ImportError:
        pass
    try:
        import trn_agent_boot.trn_boot as tb
        hook = tb._ntff_profile_via_ctypes('/opt/axon/libaxon_pjrt.so')
        if hook is None:
            return
        mod = types.ModuleType('antenv.axon_hooks')
        mod.get_axon_ntff_profile_hook = lambda: hook
        mod.set_axon_ntff_profile_hook = lambda h: None
        sys.modules['antenv.axon_hooks'] = mod
        import antenv
        antenv.axon_hooks = mod
    except Exception:
        pass


class _Stuffer:
    """Ordered pool of PE filler work, pulled by cycle budget between
    attention pieces so emission (= per-engine execution order) interleaves."""

    def __init__(self):
        self.q = []
        self.done = set()
        self.debt = 0.0

    def push(self, sid, cycles, fn):
        self.q.append((sid, cycles, fn))

    def _emit_front(self):
        sid, c, fn = self.q.pop(0)
        fn()
        self.done.add(sid)
        return c

    def pull(self, budget):
        self.debt += budget
        while self.q and self.debt > 0:
            self.debt -= self._emit_front()

    def ensure(self, sids):
        while any(s not in self.done for s in sids):
            self._emit_front()

    def drain(self):
        while self.q:
            self._emit_front()


def _build_program(causal: bool):
    import concourse.bass as bass  # noqa: F401
    from concourse import bacc
    import concourse.tile as tile
    from concourse import mybir

    F32 = mybir.dt.float32
    F16 = mybir.dt.float16
    AF = mybir.ActivationFunctionType
    MUL = mybir.AluOpType.mult
    ADD = mybir.AluOpType.add

    NT = T // NQ          # proj/attention q-chunks (4)
    NKB = T // P          # k-blocks (16)
    DB = D // P           # d-blocks (8)
    CB = CH // P          # channel blocks = head-pair blocks (2)

    nc = bacc.Bacc("TRN2", target_bir_lowering=False, debug=False)

    # x chunk-major [m][p][o*NQ], weights partition-major [p][o][c] so each
    # DMA descriptor covers a 2-4KB contiguous run (descriptor generation on
    # the trigger queue costs ~5ns/descriptor, serialized)
    xPM = nc.dram_tensor("xPM", [T // NQ, P, DB * NQ], F16, kind="ExternalInput").ap()
    wqPM = nc.dram_tensor("wqPM", [P, DB * CH], F16, kind="ExternalInput").ap()
    wkPM = nc.dram_tensor("wkPM", [P, DB * CH], F16, kind="ExternalInput").ap()
    wvPM = nc.dram_tensor("wvPM", [P, DB * CH], F16, kind="ExternalInput").ap()
    woPM = nc.dram_tensor("woPM", [P, CB * D], F16, kind="ExternalInput").ap()
    cosT = nc.dram_tensor("cosT", [P, T], F16, kind="ExternalInput").ap()
    sinS = nc.dram_tensor("sinS", [P, T], F16, kind="ExternalInput").ap()
    ident = nc.dram_tensor("ident", [P, P], F16, kind="ExternalInput").ap()
    triB = nc.dram_tensor("triB", [P, P], F16, kind="ExternalInput").ap()
    onescol = nc.dram_tensor("onescol", [P, NKB * HPC], F16, kind="ExternalInput").ap()
    out = nc.dram_tensor("out", [T, D], F16, kind="ExternalOutput").ap()

    with tile.TileContext(nc) as tc:
        with tc.tile_pool(name="singles", bufs=1) as singles, \
             tc.tile_pool(name="rope_tmp", bufs=3) as rope_pool, \
             tc.tile_pool(name="ptp", bufs=4) as pt_pool, \
             tc.tile_pool(name="obp", bufs=3) as ob_pool, \
             tc.tile_pool(name="recp", bufs=3) as rec_pool, \
             tc.tile_pool(name="bcp", bufs=3) as bc_pool, \
             tc.tile_pool(name="st_ps", bufs=2, space="PSUM") as st_pool, \
             tc.tile_pool(name="ot_ps", bufs=1, space="PSUM") as ot_pool, \
             tc.tile_pool(name="pp_ps", bufs=2, space="PSUM") as pp_pool:

            # ---- resident tiles ----
            # chunk-major x: [p][m][o][q] so each load descriptor is 4KB
            xT_sb = singles.tile([P, NT, DB, NQ], F16)
            wqT_sb = singles.tile([P, DB, CH], F16)
            wkT_sb = singles.tile([P, DB, CH], F16)
            wvT_sb = singles.tile([P, DB, CH], F16)
            woT_sb = singles.tile([P, CB, D], F16)
            cosT_sb = singles.tile([P, T], F16)
            sinS_sb = singles.tile([P, T], F16)
            ident_sb = singles.tile([P, P], F16)
            triB_sb = singles.tile([P, P], F16)
            QT_sb = singles.tile([P, CB, T], F16)
            KT_sb = singles.tile([P, CB, T], F16)
            attnT_sb = singles.tile([P, CB, T], F16)
            vaug = singles.tile([P, NKB, HPC, HD + 1], F16)
            otsb = [singles.tile([HD + 1, 2 * NT, NQ], F32, name=f"otsb_{hp}")
                    for hp in range(CB)]

            # ---- input DMAs ----
            # Triggers serialize at ~0.7us each on the issuing queue; split the
            # early-critical loads (x m0, rope tables, wv) onto the otherwise
            # idle scalar queue, the rest on sync, both in need-order.
            def load(eng, dst, src, ways, axis=0):
                n = dst.shape[axis] if axis == 0 else dst.shape[1]
                step = n // ways
                for s in range(ways):
                    sl = slice(s * step, (s + 1) * step)
                    if axis == 0:
                        eng.dma_start(dst[sl], src[sl])
                    else:
                        eng.dma_start(dst[:, sl], src[:, sl])

            pass

            def pload(eng, dst2, src2, ways):
                # partition-range split; dst2 is a plain (non-rearranged) SBUF
                # AP so subtile dependency tracking is exact
                step = P // ways
                for s in range(ways):
                    sl = slice(s * step, (s + 1) * step)
                    eng.dma_start(dst2[sl], src2[sl])

            xPMr = xPM.rearrange("m p (o q) -> m p o q", o=DB)
            wkPMr = wkPM.rearrange("p (o c) -> p o c", o=DB)
            wqPMr = wqPM.rearrange("p (o c) -> p o c", o=DB)
            wvPMr = wvPM.rearrange("p (o c) -> p o c", o=DB)
            woPMr = woPM.rearrange("p (o c) -> p o c", o=CB)

            # scalar queue: x m0, cos, sin (m0 columns first), wv
            pload(nc.scalar, xT_sb[:, 0], xPMr[0], 8)
            nc.scalar.dma_start(cosT_sb[:, 0:NQ], cosT[:, 0:NQ])
            nc.scalar.dma_start(sinS_sb[:, 0:NQ], sinS[:, 0:NQ])
            nc.scalar.dma_start(cosT_sb[:, NQ:2 * NQ], cosT[:, NQ:2 * NQ])
            nc.scalar.dma_start(sinS_sb[:, NQ:2 * NQ], sinS[:, NQ:2 * NQ])
            pload(nc.scalar, wvT_sb, wvPMr, 4)
            for m in range(2, NT):
                ms = slice(m * NQ, (m + 1) * NQ)
                nc.scalar.dma_start(cosT_sb[:, ms], cosT[:, ms])
                nc.scalar.dma_start(sinS_sb[:, ms], sinS[:, ms])
            # sync queue: weights, remaining x, constants
            pload(nc.sync, wkT_sb, wkPMr, 8)
            pload(nc.sync, wqT_sb, wqPMr, 8)
            pload(nc.sync, xT_sb[:, 1], xPMr[1], 8)
            nc.sync.dma_start(ident_sb[:], ident[:])
            nc.sync.dma_start(triB_sb[:], triB[:])
            nc.sync.dma_start(
                vaug[:, :, :, HD:HD + 1],
                onescol.rearrange("p (a b) -> p a b", a=NKB)[:, :, :, None])
            pload(nc.sync, xT_sb[:, 2], xPMr[2], 4)
            pload(nc.sync, xT_sb[:, 3], xPMr[3], 4)
            pload(nc.sync, woT_sb, woPMr, 2)

            # ---- work-chunk emitters ----
            SWAP_MASK = [i ^ 1 for i in range(32)]

            def proj_chunk(w_sb, dst_sb, cb, m, pname):
                ms = slice(m * NQ, (m + 1) * NQ)
                ps = pp_pool.tile([P, NQ], F32, tag="pp",
                                  name=f"ps_{pname}{cb}{m}")
                for o in range(DB):
                    nc.tensor.matmul(
                        ps[:],
                        w_sb[:, o, cb * P:(cb + 1) * P],
                        xT_sb[:, m, o, :],
                        start=(o == 0), stop=(o == DB - 1))
                # RoPE (pair-interleaved channels): dst = ps*cos + swap1(ps*sinS)
                nc.vector.tensor_tensor(dst_sb[:, cb, ms], ps[:],
                                        cosT_sb[:, ms], MUL)
                tmp = rope_pool.tile([P, NQ], F16, tag="tmp",
                                     name=f"tm_{pname}{cb}{m}")
                nc.vector.tensor_tensor(tmp[:], ps[:], sinS_sb[:, ms], MUL)
                tmps = rope_pool.tile([P, NQ], F16, tag="tmps",
                                      name=f"tms_{pname}{cb}{m}")
                nc.vector.stream_shuffle(tmps[:], tmp[:], SWAP_MASK)
                nc.vector.tensor_tensor(dst_sb[:, cb, ms],
                                        dst_sb[:, cb, ms], tmps[:], ADD)

            def v_chunk(i):
                ps = pp_pool.tile([P, NQ], F32, tag="pp", name=f"vps_{i}")
                vp = ps[:, :CH]
                im, ic = i // 4, (i % 4) * P
                for o in range(DB):
                    nc.tensor.matmul(
                        vp,
                        xT_sb[:, im, o, ic:ic + P],
                        wvT_sb[:, o, :],
                        start=(o == 0), stop=(o == DB - 1))
                nc.vector.tensor_copy(
                    vaug[:, i, :, 0:HD],
                    vp.rearrange("p (h d) -> p h d", h=HPC))

            def wo_chunk(i, j):
                ps = pp_pool.tile([P, NQ], F32, tag="pp", name=f"ops_{i}_{j}")
                for cb in range(CB):
                    nc.tensor.matmul(
                        ps[:],
                        attnT_sb[:, cb, i * P:(i + 1) * P],
                        woT_sb[:, cb, j * NQ:(j + 1) * NQ],
                        start=(cb == 0), stop=(cb == CB - 1))
                ob = ob_pool.tile([P, NQ], F16, tag="ob", name=f"ob_{i}_{j}")
                nc.vector.tensor_copy(ob[:], ps[:])
                ways = 2 if i >= 12 else 1
                step = P // ways
                for s in range(ways):
                    sl = slice(s * step, (s + 1) * step)
                    nc.sync.dma_start(
                        out[i * P:(i + 1) * P, j * NQ:(j + 1) * NQ][sl], ob[sl])

            stuffer = _Stuffer()

            def kb_list(qc):
                return list(range(min(NKB, (qc + 1) * (NQ // P)))) if causal \
                    else list(range(NKB))

            # ---- attention ----
            def attn_chunk(hp, qc):
                kbs = kb_list(qc)
                q0 = qc * NQ
                ot = ot_pool.tile([HD + 1, 2, NQ], F32, tag="ot",
                                  name=f"ot_{hp}_{qc}")

                def pv(kb, pt, qsl):
                    for half in range(2):
                        h = hp * 2 + half
                        nc.tensor.matmul(
                            ot[:, half, qsl:NQ],
                            vaug[:, kb, h, :],
                            pt[:, half, qsl:NQ],
                            start=(kb == kbs[0]), stop=(kb == kbs[-1]))

                pend = None
                for kb in kbs:
                    qsl = max(0, kb * P - q0) if causal else 0
                    diag = causal and kb * P >= q0
                    st = st_pool.tile([P, 2, NQ], F32, tag="st",
                                      name=f"st_{hp}_{qc}_{kb}")
                    for half in range(2):
                        hb = half * HD
                        nc.tensor.matmul(
                            st[:, half, qsl:NQ],
                            KT_sb[hb:hb + HD, hp, kb * P:(kb + 1) * P],
                            QT_sb[hb:hb + HD, hp, q0 + qsl:q0 + NQ],
                            start=True, stop=not diag)
                        if diag:
                            # causal mask: add -30000 strictly below the
                            # diagonal so exp underflows those to zero
                            nc.tensor.matmul(
                                st[:, half, qsl:qsl + P],
                                ident_sb[:],
                                triB_sb[:],
                                start=False, stop=True)
                    pt = pt_pool.tile([P, 2, NQ], F16, tag="pt",
                                      name=f"pt_{hp}_{qc}_{kb}")
                    sf = st.rearrange("p a b -> p (a b)")
                    pf = pt.rearrange("p a b -> p (a b)")
                    # one exp covers both halves; the uncomputed middle
                    # columns of diagonal blocks are never read downstream
                    nc.scalar.activation(pf[:, qsl:2 * NQ], sf[:, qsl:2 * NQ],
                                         AF.Exp, scale=float(HD) ** -0.5)
                    if pend is not None:
                        stuffer.pull(KB_BUDGET)
                        pv(*pend)
                    pend = (kb, pt, qsl)
                stuffer.pull(KB_BUDGET)
                pv(*pend)

                # softmax stats: stage to SBUF (frees PSUM), fast-reciprocal of
                # the ones-row, gpsimd broadcast + normalize multiply into attnT
                i0 = qc * 2
                nc.vector.tensor_copy(otsb[hp][:, i0:i0 + 2, :], ot[:, :, :])
                # sums live on partition 64; hop them to partition 0 by DMA so
                # the custom-DVE reciprocal sees matching in/out partitions
                sums = rec_pool.tile([1, 2, NQ], F32, tag="sums",
                                     name=f"sums_{hp}_{qc}")
                nc.sync.dma_start(sums[:], otsb[hp][HD:HD + 1, i0:i0 + 2, :])
                for half in range(2):
                    rec = rec_pool.tile([1, NQ], F32, tag="rec",
                                        name=f"rec_{hp}_{qc}_{half}")
                    nc.vector.reciprocal_approx_fast(
                        rec[:], sums[:, half, :])
                    bc = bc_pool.tile([HD, NQ], F32, tag="bc",
                                      name=f"bc_{hp}_{qc}_{half}")
                    nc.gpsimd.partition_broadcast(bc[:], rec[:])
                    nc.vector.tensor_tensor(
                        attnT_sb[half * HD:(half + 1) * HD, hp, q0:q0 + NQ],
                        otsb[hp][0:HD, i0 + half, :], bc[:], MUL)

            # ---- emission schedule ----
            # upfront: hp0 m0 projections + V level 0
            proj_chunk(wkT_sb, KT_sb, 0, 0, "k")
            proj_chunk(wqT_sb, QT_sb, 0, 0, "q")
            for i in range(4):
                v_chunk(i)

            stuffer.push(("k", 1, 0), 4096,
                         lambda: proj_chunk(wkT_sb, KT_sb, 1, 0, "k"))
            stuffer.push(("q", 1, 0), 4096,
                         lambda: proj_chunk(wqT_sb, QT_sb, 1, 0, "q"))
            for lvl in range(1, NT):
                for (nm, w_sb, d_sb, cb) in (("k", wkT_sb, KT_sb, 0),
                                             ("q", wqT_sb, QT_sb, 0)):
                    stuffer.push(
                        (nm, cb, lvl), 4096,
                        lambda w=w_sb, d=d_sb, c=cb, m=lvl, n=nm:
                            proj_chunk(w, d, c, m, n))
                for i in range(lvl * 4, lvl * 4 + 4):
                    stuffer.push(("v", i), 2048, lambda i=i: v_chunk(i))
                for (nm, w_sb, d_sb, cb) in (("k", wkT_sb, KT_sb, 1),
                                             ("q", wqT_sb, QT_sb, 1)):
                    stuffer.push(
                        (nm, cb, lvl), 4096,
                        lambda w=w_sb, d=d_sb, c=cb, m=lvl, n=nm:
                            proj_chunk(w, d, c, m, n))

            for qc in range(NT):
                need0 = [("k", 0, m) for m in range(1, qc + 1)] + \
                        [("q", 0, m) for m in range(1, qc + 1)] + \
                        [("v", i) for i in range(4, (qc + 1) * 4)]
                stuffer.ensure(need0)
                attn_chunk(0, qc)
                need1 = [("k", 1, m) for m in range(qc + 1)] + \
                        [("q", 1, m) for m in range(qc + 1)]
                stuffer.ensure(need1)
                attn_chunk(1, qc)
                # wo chunks of this q-range now have both head-pairs normalized
                for i in range(qc * 4, qc * 4 + 4):
                    for j in range(D // NQ):
                        stuffer.push(("wo", i, j), 1024,
                                     lambda i=i, j=j: wo_chunk(i, j))
            stuffer.drain()

    nc.compile()
    return nc


def _get_program(causal: bool):
    key = ("causal" if causal else "full")
    if key not in _prog_cache:
        _prog_cache[key] = _build_program(causal)
    return _prog_cache[key]


def _mask_kind(mask):
    m = np.asarray(mask)
    if m.ndim == 4:
        m = m[0, 0]
    if (m != 0).all():
        return False  # full attention
    trilm = np.tril(np.ones((m.shape[0], m.shape[1]), dtype=m.dtype))
    if np.array_equal(m, trilm):
        return True
    raise NotImplementedError("mask is neither all-ones nor causal tril")


# per-head channel permutation: pair-interleave so RoPE partner = c ^ 1.
# new position 2i  <- old channel i       (i < 32)
# new position 2i+1<- old channel i + 32
_PAIRPERM = np.empty(HD, dtype=np.int64)
_PAIRPERM[0::2] = np.arange(HD // 2)
_PAIRPERM[1::2] = np.arange(HD // 2) + HD // 2


def _make_in_maps(x, cos, sin, wq, wk, wv, wo):
    x = np.asarray(x, dtype=np.float32)
    cos = np.asarray(cos, dtype=np.float32)
    sin = np.asarray(sin, dtype=np.float32)
    wq = np.asarray(wq, dtype=np.float32)
    wk = np.asarray(wk, dtype=np.float32)
    wv = np.asarray(wv, dtype=np.float32)
    wo = np.asarray(wo, dtype=np.float32)

    # RoPE tables in the pair-interleaved layout [128ch = 2 heads, T].
    # cosT[2i] = cosT[2i+1] = cos[t, i]; sinS[2i] = +sin[t, i] (the term
    # destined for 2i+1), sinS[2i+1] = -sin[t, i] (destined for 2i).
    half = np.arange(HD // 2)
    cos_half = cos[:T, :HD // 2]                     # [T, 32]
    sin_half = sin[:T, :HD // 2]
    cos_il = np.empty((HD, T), dtype=np.float32)
    sin_il = np.empty((HD, T), dtype=np.float32)
    cos_il[0::2] = cos_half.T[half]
    cos_il[1::2] = cos_half.T[half]
    sin_il[0::2] = sin_half.T[half]
    sin_il[1::2] = -sin_half.T[half]
    cosT16 = np.ascontiguousarray(
        np.tile(cos_il, (2, 1)).astype(np.float16))   # [128, T]
    sinS16 = np.ascontiguousarray(
        np.tile(sin_il, (2, 1)).astype(np.float16))
    identm = np.eye(P, dtype=np.float16)
    DBH = D // P
    NTH = T // NQ
    # x chunk-major: [m][p][o*NQ] with element (m,p,o,q) = x[t=m*NQ+q, d=o*P+p]
    xpm = []
    for b_ in range(B):
        xt = x[b_].T.astype(np.float16)                    # [D, T]
        v4 = xt.reshape(DBH, P, NTH, NQ).transpose(2, 1, 0, 3)   # [m][p][o][q]
        xpm.append(np.ascontiguousarray(v4.reshape(NTH, P, DBH * NQ)))
    triBm = np.ascontiguousarray(
        (np.tril(np.ones((P, P), np.float32), -1) * -30000.0).astype(np.float16))
    ones = np.ones((P, (T // P) * HPC), dtype=np.float16)

    # global channel permutation for the q/k weight rows of this core's slice
    qkperm = np.concatenate([h * HD + _PAIRPERM for h in range(HPC)])

    in_maps = []
    for core in range(NCORES):
        b = core // GROUPS
        g = core % GROUPS
        c0 = g * CH
        wq_s = wq[c0:c0 + CH, :][qkperm]
        wk_s = wk[c0:c0 + CH, :][qkperm]

        def wpm(w):      # [D, CH] -> [p][o][c] flattened
            return np.ascontiguousarray(
                w.T.astype(np.float16).reshape(DBH, P, CH).transpose(1, 0, 2)
                .reshape(P, DBH * CH))

        xb = xpm[b]
        in_maps.append({
            "xPM": xb,
            "wqPM": wpm(wq_s),
            "wkPM": wpm(wk_s),
            "wvPM": wpm(wv[c0:c0 + CH, :]),
            "woPM": np.ascontiguousarray(
                wo[:, c0:c0 + CH].T.astype(np.float16)
                .reshape(2, P, D).transpose(1, 0, 2).reshape(P, 2 * D)),
            "cosT": cosT16,
            "sinS": sinS16,
            "ident": identm,
            "triB": triBm,
            "onescol": ones,
        })
    return in_maps


def _run(inputs, trace=False):
    from concourse import bass_utils
    causal = _mask_kind(inputs["mask"])
    nc = _get_program(causal)
    in_maps = _make_in_maps(
        inputs["x"], inputs["cos"], inputs["sin"],
        inputs["wq"], inputs["wk"], inputs["wv"], inputs["wo"])
    if trace:
        _install_ntff_shim()
    res = bass_utils.run_bass_kernel_spmd(
        nc, in_maps, core_ids=list(range(NCORES)), trace=trace)
    outs = [r["out"] for r in res.results]
    full = np.empty((B, T, D), dtype=np.float32)
    for b in range(B):
        acc = outs[b * GROUPS].astype(np.float32)
        for g in range(1, GROUPS):
            acc += outs[b * GROUPS + g].astype(np.float32)
        full[b] = acc
    return full, res


def kernel(**inputs):
    full, _ = _run(inputs, trace=False)
    return full


def kernel_profiled(**inputs):
    """Like kernel() but with NTFF tracing; returns (out, BassKernelResults)."""
    return _run(inputs, trace=True)


# revision 14
# speedup vs baseline: 1.1489x; 1.1489x over previous
"""Trainium2 Bass kernel for nn_Attention_47682726920277.

Causal multi-head attention with RoPE:
  q/k/v = x @ w{q,k,v}.T ; RoPE(q, k) ; att = softmax(mask(q k^T / 8)) ; out = (att v) @ wo.T
Shapes: x [2, 2048, 1024], 16 heads of dim 64, fp32.

Sharding (8 cores): data-parallel over batch (2) x tensor-parallel over heads (4 per
core). Each core computes its 4 heads' attention and a partial out via its wo row
block; the final all-reduce is the host-side sum of the 4 partials per batch.

Design notes:
- The PE matmul stream (~115us of columns at 2.4GHz) and the scalar engine's
  softmax-exp stream (~66us) are the two long poles; projection/V/wo matmuls
  are interleaved between each k-block's score and PV matmuls at ~1.1k-cycle
  granularity so the PE never idles (idle also drops the PE to its half-rate
  DVFS p-state).
- Q/K channels are stored pair-interleaved per head ([0,32,1,33,...] of the
  natural order) so RoPE's rotate-half partner is the adjacent channel; the
  partner term is then a single vector stream_shuffle (a per-32-lane-quadrant
  crossbar) instead of 4 partition-window ops. Scores are invariant to the
  permutation since q and k share it.
- DMA triggers cost ~0.7us each, serialized per issuing queue: input loads are
  split between the sync and scalar queues and kept coarse; wo outputs are one
  trigger each.
- Scalar runs exp exclusively (one activation table, no swaps). Softmax
  reciprocals use the DVE fast-approx reciprocal; the per-column denominator
  broadcast and the normalize multiply run on gpsimd.
"""
import sys
import types
import numpy as np

B = 2
T = 2048
D = 1024
H = 16
HD = 64
NCORES = 8
GROUPS = NCORES // B          # head-groups per batch
HPC = H // GROUPS             # heads per core = 4
CH = HPC * HD                 # channels per core = 256
NQ = 512                      # PSUM bank width (fp32)
P = 128

KB_BUDGET = 1100              # PE cycles of stuffing pulled per attention k-block

_prog_cache = {}


def _install_ntff_shim():
    """The agent image's antenv lacks axon_hooks; inject it so trace=True works."""
    try:
        import antenv.axon_hooks  # noqa: F401
        return
    except ImportError:
        pass
    try:
        import trn_agent_boot.trn_boot as tb
        hook = tb._ntff_profile_via_ctypes('/opt/axon/libaxon_pjrt.so')
        if hook is None:
            return
        mod = types.ModuleType('antenv.axon_hooks')
        mod.get_axon_ntff_profile_hook = lambda: hook
        mod.set_axon_ntff_profile_hook = lambda h: None
        sys.modules['antenv.axon_hooks'] = mod
        import antenv
        antenv.axon_hooks = mod
    except Exception:
        pass


class _Stuffer:
    """Ordered pool of PE filler work, pulled by cycle budget between
    attention pieces so emission (= per-engine execution order) interleaves."""

    def __init__(self):
        self.q = []
        self.done = set()
        self.debt = 0.0

    def push(self, sid, cycles, fn):
        self.q.append((sid, cycles, fn))

    def _emit_front(self):
        sid, c, fn = self.q.pop(0)
        fn()
        self.done.add(sid)
        return c

    def pull(self, budget):
        self.debt += budget
        while self.q and self.debt > 0:
            self.debt -= self._emit_front()

    def ensure(self, sids):
        while any(s not in self.done for s in sids):
            self._emit_front()

    def drain(self):
        while self.q:
            self._emit_front()


def _build_program(causal: bool):
    import concourse.bass as bass  # noqa: F401
    from concourse import bacc
    import concourse.tile as tile
    from concourse import mybir

    F32 = mybir.dt.float32
    F16 = mybir.dt.float16
    AF = mybir.ActivationFunctionType
    MUL = mybir.AluOpType.mult
    ADD = mybir.AluOpType.add

    NT = T // NQ          # proj/attention q-chunks (4)
    NKB = T // P          # k-blocks (16)
    DB = D // P           # d-blocks (8)
    CB = CH // P          # channel blocks = head-pair blocks (2)

    nc = bacc.Bacc("TRN2", target_bir_lowering=False, debug=False)

    xT = nc.dram_tensor("xT", [D, T], F16, kind="ExternalInput").ap()
    wqT = nc.dram_tensor("wqT", [D, CH], F16, kind="ExternalInput").ap()
    wkT = nc.dram_tensor("wkT", [D, CH], F16, kind="ExternalInput").ap()
    wvT = nc.dram_tensor("wvT", [D, CH], F16, kind="ExternalInput").ap()
    woT = nc.dram_tensor("woT", [CH, D], F16, kind="ExternalInput").ap()
    cosT = nc.dram_tensor("cosT", [P, T], F16, kind="ExternalInput").ap()
    sinS = nc.dram_tensor("sinS", [P, T], F16, kind="ExternalInput").ap()
    ident = nc.dram_tensor("ident", [P, P], F16, kind="ExternalInput").ap()
    triB = nc.dram_tensor("triB", [P, P], F16, kind="ExternalInput").ap()
    onescol = nc.dram_tensor("onescol", [P, NKB * HPC], F16, kind="ExternalInput").ap()
    out = nc.dram_tensor("out", [T, D], F16, kind="ExternalOutput").ap()

    with tile.TileContext(nc) as tc:
        with tc.tile_pool(name="singles", bufs=1) as singles, \
             tc.tile_pool(name="rope_tmp", bufs=3) as rope_pool, \
             tc.tile_pool(name="ptp", bufs=4) as pt_pool, \
             tc.tile_pool(name="obp", bufs=3) as ob_pool, \
             tc.tile_pool(name="recp", bufs=3) as rec_pool, \
             tc.tile_pool(name="bcp", bufs=3) as bc_pool, \
             tc.tile_pool(name="st_ps", bufs=2, space="PSUM") as st_pool, \
             tc.tile_pool(name="ot_ps", bufs=1, space="PSUM") as ot_pool, \
             tc.tile_pool(name="pp_ps", bufs=2, space="PSUM") as pp_pool:

            # ---- resident tiles ----
            xT_sb = singles.tile([P, DB, T], F16)
            wqT_sb = singles.tile([P, DB, CH], F16)
            wkT_sb = singles.tile([P, DB, CH], F16)
            wvT_sb = singles.tile([P, DB, CH], F16)
            woT_sb = singles.tile([P, CB, D], F16)
            cosT_sb = singles.tile([P, T], F16)
            sinS_sb = singles.tile([P, T], F16)
            ident_sb = singles.tile([P, P], F16)
            triB_sb = singles.tile([P, P], F16)
            QT_sb = singles.tile([P, CB, T], F16)
            KT_sb = singles.tile([P, CB, T], F16)
            attnT_sb = singles.tile([P, CB, T], F16)
            vaug = singles.tile([P, NKB, HPC, HD + 1], F16)
            otsb = [singles.tile([HD + 1, 2 * NT, NQ], F32, name=f"otsb_{hp}")
                    for hp in range(CB)]

            # ---- input DMAs ----
            # Triggers serialize at ~0.7us each on the issuing queue; split the
            # early-critical loads (x m0, rope tables, wv) onto the otherwise
            # idle scalar queue, the rest on sync, both in need-order.
            def load(eng, dst, src, ways, axis=0):
                n = dst.shape[axis] if axis == 0 else dst.shape[1]
                step = n // ways
                for s in range(ways):
                    sl = slice(s * step, (s + 1) * step)
                    if axis == 0:
                        eng.dma_start(dst[sl], src[sl])
                    else:
                        eng.dma_start(dst[:, sl], src[:, sl])

            xTr = xT.rearrange("(o p) t -> p o t", p=P)
            wkr = wkT.rearrange("(o p) c -> p o c", p=P)
            wqr = wqT.rearrange("(o p) c -> p o c", p=P)
            wvr = wvT.rearrange("(o p) c -> p o c", p=P)
            wor = woT.rearrange("(o p) c -> p o c", p=P)

            # scalar queue: x m0, cos, sin (column-split, m0 columns first), wv
            load(nc.scalar, xT_sb[:, :, 0:NQ], xTr[:, :, 0:NQ], 8, axis=1)
            nc.scalar.dma_start(cosT_sb[:, 0:NQ], cosT[:, 0:NQ])
            nc.scalar.dma_start(sinS_sb[:, 0:NQ], sinS[:, 0:NQ])
            nc.scalar.dma_start(cosT_sb[:, NQ:2 * NQ], cosT[:, NQ:2 * NQ])
            nc.scalar.dma_start(sinS_sb[:, NQ:2 * NQ], sinS[:, NQ:2 * NQ])
            load(nc.scalar, wvT_sb, wvr, 4, axis=1)
            for m in range(2, NT):
                ms = slice(m * NQ, (m + 1) * NQ)
                nc.scalar.dma_start(cosT_sb[:, ms], cosT[:, ms])
                nc.scalar.dma_start(sinS_sb[:, ms], sinS[:, ms])
            # sync queue: weights, remaining x, constants
            load(nc.sync, wkT_sb, wkr, 8, axis=1)
            load(nc.sync, wqT_sb, wqr, 8, axis=1)
            load(nc.sync, xT_sb[:, :, NQ:2 * NQ], xTr[:, :, NQ:2 * NQ], 8, axis=1)
            nc.sync.dma_start(ident_sb[:], ident[:])
            nc.sync.dma_start(triB_sb[:], triB[:])
            nc.sync.dma_start(
                vaug[:, :, :, HD:HD + 1],
                onescol.rearrange("p (a b) -> p a b", a=NKB)[:, :, :, None])
            load(nc.sync, xT_sb[:, :, 2 * NQ:3 * NQ], xTr[:, :, 2 * NQ:3 * NQ], 4, axis=1)
            load(nc.sync, xT_sb[:, :, 3 * NQ:4 * NQ], xTr[:, :, 3 * NQ:4 * NQ], 4, axis=1)
            load(nc.sync, woT_sb, wor, 2, axis=1)

            # ---- work-chunk emitters ----
            SWAP_MASK = [i ^ 1 for i in range(32)]

            def proj_chunk(w_sb, dst_sb, cb, m, pname):
                ms = slice(m * NQ, (m + 1) * NQ)
                ps = pp_pool.tile([P, NQ], F32, tag="pp",
                                  name=f"ps_{pname}{cb}{m}")
                for o in range(DB):
                    nc.tensor.matmul(
                        ps[:],
                        w_sb[:, o, cb * P:(cb + 1) * P],
                        xT_sb[:, o, ms],
                        start=(o == 0), stop=(o == DB - 1))
                # RoPE (pair-interleaved channels): dst = ps*cos + swap1(ps*sinS)
                nc.vector.tensor_tensor(dst_sb[:, cb, ms], ps[:],
                                        cosT_sb[:, ms], MUL)
                tmp = rope_pool.tile([P, NQ], F16, tag="tmp",
                                     name=f"tm_{pname}{cb}{m}")
                nc.vector.tensor_tensor(tmp[:], ps[:], sinS_sb[:, ms], MUL)
                tmps = rope_pool.tile([P, NQ], F16, tag="tmps",
                                      name=f"tms_{pname}{cb}{m}")
                nc.vector.stream_shuffle(tmps[:], tmp[:], SWAP_MASK)
                nc.vector.tensor_tensor(dst_sb[:, cb, ms],
                                        dst_sb[:, cb, ms], tmps[:], ADD)

            def v_chunk(i):
                ps = pp_pool.tile([P, NQ], F32, tag="pp", name=f"vps_{i}")
                vp = ps[:, :CH]
                for o in range(DB):
                    nc.tensor.matmul(
                        vp,
                        xT_sb[:, o, i * P:(i + 1) * P],
                        wvT_sb[:, o, :],
                        start=(o == 0), stop=(o == DB - 1))
                nc.vector.tensor_copy(
                    vaug[:, i, :, 0:HD],
                    vp.rearrange("p (h d) -> p h d", h=HPC))

            def wo_chunk(i, j):
                ps = pp_pool.tile([P, NQ], F32, tag="pp", name=f"ops_{i}_{j}")
                for cb in range(CB):
                    nc.tensor.matmul(
                        ps[:],
                        attnT_sb[:, cb, i * P:(i + 1) * P],
                        woT_sb[:, cb, j * NQ:(j + 1) * NQ],
                        start=(cb == 0), stop=(cb == CB - 1))
                ob = ob_pool.tile([P, NQ], F16, tag="ob", name=f"ob_{i}_{j}")
                nc.vector.tensor_copy(ob[:], ps[:])
                ways = 2 if i >= 12 else 1
                step = P // ways
                for s in range(ways):
                    sl = slice(s * step, (s + 1) * step)
                    nc.sync.dma_start(
                        out[i * P:(i + 1) * P, j * NQ:(j + 1) * NQ][sl], ob[sl])

            stuffer = _Stuffer()

            def kb_list(qc):
                return list(range(min(NKB, (qc + 1) * (NQ // P)))) if causal \
                    else list(range(NKB))

            # ---- attention ----
            def attn_chunk(hp, qc):
                kbs = kb_list(qc)
                q0 = qc * NQ
                ot = ot_pool.tile([HD + 1, 2, NQ], F32, tag="ot",
                                  name=f"ot_{hp}_{qc}")

                def pv(kb, pt, qsl):
                    for half in range(2):
                        h = hp * 2 + half
                        nc.tensor.matmul(
                            ot[:, half, qsl:NQ],
                            vaug[:, kb, h, :],
                            pt[:, half, qsl:NQ],
                            start=(kb == kbs[0]), stop=(kb == kbs[-1]))

                pend = None
                for kb in kbs:
                    qsl = max(0, kb * P - q0) if causal else 0
                    diag = causal and kb * P >= q0
                    st = st_pool.tile([P, 2, NQ], F32, tag="st",
                                      name=f"st_{hp}_{qc}_{kb}")
                    for half in range(2):
                        hb = half * HD
                        nc.tensor.matmul(
                            st[:, half, qsl:NQ],
                            KT_sb[hb:hb + HD, hp, kb * P:(kb + 1) * P],
                            QT_sb[hb:hb + HD, hp, q0 + qsl:q0 + NQ],
                            start=True, stop=not diag)
                        if diag:
                            # causal mask: add -30000 strictly below the
                            # diagonal so exp underflows those to zero
                            nc.tensor.matmul(
                                st[:, half, qsl:qsl + P],
                                ident_sb[:],
                                triB_sb[:],
                                start=False, stop=True)
                    pt = pt_pool.tile([P, 2, NQ], F16, tag="pt",
                                      name=f"pt_{hp}_{qc}_{kb}")
                    sf = st.rearrange("p a b -> p (a b)")
                    pf = pt.rearrange("p a b -> p (a b)")
                    # one exp covers both halves; the uncomputed middle
                    # columns of diagonal blocks are never read downstream
                    nc.scalar.activation(pf[:, qsl:2 * NQ], sf[:, qsl:2 * NQ],
                                         AF.Exp, scale=float(HD) ** -0.5)
                    if pend is not None:
                        stuffer.pull(KB_BUDGET)
                        pv(*pend)
                    pend = (kb, pt, qsl)
                stuffer.pull(KB_BUDGET)
                pv(*pend)

                # softmax stats: stage to SBUF (frees PSUM), fast-reciprocal of
                # the ones-row, gpsimd broadcast + normalize multiply into attnT
                i0 = qc * 2
                nc.vector.tensor_copy(otsb[hp][:, i0:i0 + 2, :], ot[:, :, :])
                # sums live on partition 64; hop them to partition 0 by DMA so
                # the custom-DVE reciprocal sees matching in/out partitions
                sums = rec_pool.tile([1, 2, NQ], F32, tag="sums",
                                     name=f"sums_{hp}_{qc}")
                nc.sync.dma_start(sums[:], otsb[hp][HD:HD + 1, i0:i0 + 2, :])
                for half in range(2):
                    rec = rec_pool.tile([1, NQ], F32, tag="rec",
                                        name=f"rec_{hp}_{qc}_{half}")
                    nc.vector.reciprocal_approx_fast(
                        rec[:], sums[:, half, :])
                    bc = bc_pool.tile([HD, NQ], F32, tag="bc",
                                      name=f"bc_{hp}_{qc}_{half}")
                    nc.gpsimd.partition_broadcast(bc[:], rec[:])
                    nc.vector.tensor_tensor(
                        attnT_sb[half * HD:(half + 1) * HD, hp, q0:q0 + NQ],
                        otsb[hp][0:HD, i0 + half, :], bc[:], MUL)

            # ---- emission schedule ----
            # upfront: hp0 m0 projections + V level 0
            proj_chunk(wkT_sb, KT_sb, 0, 0, "k")
            proj_chunk(wqT_sb, QT_sb, 0, 0, "q")
            for i in range(4):
                v_chunk(i)

            stuffer.push(("k", 1, 0), 4096,
                         lambda: proj_chunk(wkT_sb, KT_sb, 1, 0, "k"))
            stuffer.push(("q", 1, 0), 4096,
                         lambda: proj_chunk(wqT_sb, QT_sb, 1, 0, "q"))
            for lvl in range(1, NT):
                for (nm, w_sb, d_sb, cb) in (("k", wkT_sb, KT_sb, 0),
                                             ("q", wqT_sb, QT_sb, 0)):
                    stuffer.push(
                        (nm, cb, lvl), 4096,
                        lambda w=w_sb, d=d_sb, c=cb, m=lvl, n=nm:
                            proj_chunk(w, d, c, m, n))
                for i in range(lvl * 4, lvl * 4 + 4):
                    stuffer.push(("v", i), 2048, lambda i=i: v_chunk(i))
                for (nm, w_sb, d_sb, cb) in (("k", wkT_sb, KT_sb, 1),
                                             ("q", wqT_sb, QT_sb, 1)):
                    stuffer.push(
                        (nm, cb, lvl), 4096,
                        lambda w=w_sb, d=d_sb, c=cb, m=lvl, n=nm:
                            proj_chunk(w, d, c, m, n))

            for qc in range(NT):
                need0 = [("k", 0, m) for m in range(1, qc + 1)] + \
                        [("q", 0, m) for m in range(1, qc + 1)] + \
                        [("v", i) for i in range(4, (qc + 1) * 4)]
                stuffer.ensure(need0)
                attn_chunk(0, qc)
                need1 = [("k", 1, m) for m in range(qc + 1)] + \
                        [("q", 1, m) for m in range(qc + 1)]
                stuffer.ensure(need1)
                attn_chunk(1, qc)
                # wo chunks of this q-range now have both head-pairs normalized
                for i in range(qc * 4, qc * 4 + 4):
                    for j in range(D // NQ):
                        stuffer.push(("wo", i, j), 1024,
                                     lambda i=i, j=j: wo_chunk(i, j))
            stuffer.drain()

    nc.compile()
    return nc


def _get_program(causal: bool):
    key = ("causal" if causal else "full")
    if key not in _prog_cache:
        _prog_cache[key] = _build_program(causal)
    return _prog_cache[key]


def _mask_kind(mask):
    m = np.asarray(mask)
    if m.ndim == 4:
        m = m[0, 0]
    if (m != 0).all():
        return False  # full attention
    trilm = np.tril(np.ones((m.shape[0], m.shape[1]), dtype=m.dtype))
    if np.array_equal(m, trilm):
        return True
    raise NotImplementedError("mask is neither all-ones nor causal tril")


# per-head channel permutation: pair-interleave so RoPE partner = c ^ 1.
# new position 2i  <- old channel i       (i < 32)
# new position 2i+1<- old channel i + 32
_PAIRPERM = np.empty(HD, dtype=np.int64)
_PAIRPERM[0::2] = np.arange(HD // 2)
_PAIRPERM[1::2] = np.arange(HD // 2) + HD // 2


def _make_in_maps(x, cos, sin, wq, wk, wv, wo):
    x = np.asarray(x, dtype=np.float32)
    cos = np.asarray(cos, dtype=np.float32)
    sin = np.asarray(sin, dtype=np.float32)
    wq = np.asarray(wq, dtype=np.float32)
    wk = np.asarray(wk, dtype=np.float32)
    wv = np.asarray(wv, dtype=np.float32)
    wo = np.asarray(wo, dtype=np.float32)

    # RoPE tables in the pair-interleaved layout [128ch = 2 heads, T].
    # cosT[2i] = cosT[2i+1] = cos[t, i]; sinS[2i] = +sin[t, i] (the term
    # destined for 2i+1), sinS[2i+1] = -sin[t, i] (destined for 2i).
    half = np.arange(HD // 2)
    cos_half = cos[:T, :HD // 2]                     # [T, 32]
    sin_half = sin[:T, :HD // 2]
    cos_il = np.empty((HD, T), dtype=np.float32)
    sin_il = np.empty((HD, T), dtype=np.float32)
    cos_il[0::2] = cos_half.T[half]
    cos_il[1::2] = cos_half.T[half]
    sin_il[0::2] = sin_half.T[half]
    sin_il[1::2] = -sin_half.T[half]
    cosT16 = np.ascontiguousarray(
        np.tile(cos_il, (2, 1)).astype(np.float16))   # [128, T]
    sinS16 = np.ascontiguousarray(
        np.tile(sin_il, (2, 1)).astype(np.float16))
    identm = np.eye(P, dtype=np.float16)
    triBm = np.ascontiguousarray(
        (np.tril(np.ones((P, P), np.float32), -1) * -30000.0).astype(np.float16))
    ones = np.ones((P, (T // P) * HPC), dtype=np.float16)

    # global channel permutation for the q/k weight rows of this core's slice
    qkperm = np.concatenate([h * HD + _PAIRPERM for h in range(HPC)])

    in_maps = []
    for core in range(NCORES):
        b = core // GROUPS
        g = core % GROUPS
        c0 = g * CH
        wq_s = wq[c0:c0 + CH, :][qkperm]
        wk_s = wk[c0:c0 + CH, :][qkperm]
        in_maps.append({
            "xT": np.ascontiguousarray(x[b].T.astype(np.float16)),          # [D, T]
            "wqT": np.ascontiguousarray(wq_s.T.astype(np.float16)),
            "wkT": np.ascontiguousarray(wk_s.T.astype(np.float16)),
            "wvT": np.ascontiguousarray(wv[c0:c0 + CH, :].T.astype(np.float16)),
            "woT": np.ascontiguousarray(wo[:, c0:c0 + CH].T.astype(np.float16)),
            "cosT": cosT16,
            "sinS": sinS16,
            "ident": identm,
            "triB": triBm,
            "onescol": ones,
        })
    return in_maps


def _run(inputs, trace=False):
    from concourse import bass_utils
    causal = _mask_kind(inputs["mask"])
    nc = _get_program(causal)
    in_maps = _make_in_maps(
        inputs["x"], inputs["cos"], inputs["sin"],
        inputs["wq"], inputs["wk"], inputs["wv"], inputs["wo"])
    if trace:
        _install_ntff_shim()
    res = bass_utils.run_bass_kernel_spmd(
        nc, in_maps, core_ids=list(range(NCORES)), trace=trace)
    outs = [r["out"] for r in res.results]
    full = np.empty((B, T, D), dtype=np.float32)
    for b in range(B):
        acc = outs[b * GROUPS].astype(np.float32)
        for g in range(1, GROUPS):
            acc += outs[b * GROUPS + g].astype(np.float32)
        full[b] = acc
    return full, res


def kernel(**inputs):
    full, _ = _run(inputs, trace=False)
    return full


def kernel_profiled(**inputs):
    """Like kernel() but with NTFF tracing; returns (out, BassKernelResults)."""
    return _run(inputs, trace=True)
